# revision 31
# baseline (speedup 1.0000x reference)
"""Trainium2 Bass kernel for nn_MultiHeadedAttention_53626961658052.

Full-input contract: kernel(**inputs) takes the unsharded numpy inputs and
returns the full outputs (mean_x [4,2048,64], q [4,16,2048,64]) as a tuple,
matching the reference.

Sharding: 8 cores = 4 batches x 2 head-halves. Core c handles batch c//2 and
heads (c%2)*8 .. (c%2)*8+8. Per core:
  - query/key/weights are cast to bf16 and transposed on the PE (the
    contraction dim must sit on SBUF partitions); projections accumulate in
    fp32 PSUM with fp32 bias, so q comes out at ~2e-3;
  - scores^T = k_h^T q_h per head as two K=64 matmuls row-packed into the
    128x128 PE array (tile_position); exp on the scalar engine straight from
    PSUM (scale=1/8 fused; max-subtraction skipped: scores are in [-10, 11]);
  - x^T = [v | 16]^T @ p^T with M=65 matmuls (ones column -> 16*rowsum,
    folding the /16 head-mean into the reciprocal);
  - x^T is transposed back on the PE, normalized and accumulated on DVE.

Scheduling: the kernel is scalar-engine(exp)-bound in steady state. The Q
projection for s-chunk sc+1 and the deferred normalize work are chopped into
small items and threaded through the attention jc-loops; the Q projection for
sc0 and the Wq transposes are likewise threaded through the K projection, so
the serial prefix is just value/Wk staging plus the K projection itself.
"""

import numpy as np

import concourse.bass as bass
import concourse.mybir as mybir
import concourse.tile as tile
from concourse import bacc
from concourse.bass_utils import run_bass_kernel_spmd
from concourse.masks import make_identity
from contextlib import ExitStack
from collections import deque

F32 = mybir.dt.float32
F32R = mybir.dt.float32r
BF16 = mybir.dt.bfloat16
Exp = mybir.ActivationFunctionType.Exp
MUL = mybir.AluOpType.mult
ADD = mybir.AluOpType.add

S = 2048
D = 1024
M = 512          # head-dim columns per core = 8 heads * 64
NHEAD = 8
NPAIR = 4
DK = 64

_built = None


def _build():
    nc = bacc.Bacc(None, target_bir_lowering=False)
    query = nc.dram_tensor("query", [S, D], F32, kind="ExternalInput")
    key = nc.dram_tensor("key", [S, D], F32, kind="ExternalInput")
    value = nc.dram_tensor("value", [DK, S], F32, kind="ExternalInput")
    wq = nc.dram_tensor("wq", [M, D], F32, kind="ExternalInput")
    wk = nc.dram_tensor("wk", [M, D], F32, kind="ExternalInput")
    bq = nc.dram_tensor("bq", [M], F32, kind="ExternalInput")
    bk = nc.dram_tensor("bk", [M], F32, kind="ExternalInput")
    qout = nc.dram_tensor("qout", [M, S], F32, kind="ExternalOutput")
    xout = nc.dram_tensor("xout", [S, DK], F32, kind="ExternalOutput")

    with tile.TileContext(nc) as tc, ExitStack() as ctx:
        const = ctx.enter_context(tc.tile_pool(name="const", bufs=1))

        ident_f = const.tile([128, 128], F32)
        make_identity(nc, ident_f)
        ident_b = const.tile([128, 128], BF16)
        make_identity(nc, ident_b)

        # --- kick off every input load up front ---
        vstage = const.tile([DK, S], F32)
        nc.sync.dma_start(out=vstage, in_=value[:, :])
        wk_st = const.tile([128, 4, D], BF16)
        nc.gpsimd.dma_start(out=wk_st,
                            in_=wk[:, :].rearrange("(c p) d -> p c d", p=128))
        kin_pool = ctx.enter_context(tc.tile_pool(name="kin", bufs=8))

        def k_load(sc, nsplit=2):
            w = D // nsplit
            pieces = []
            for h in range(nsplit):
                kin = kin_pool.tile([128, 4, 512], BF16, tag="kin")
                nc.gpsimd.dma_start(
                    out=kin[:, :, 0:w] if w < 512 else kin,
                    in_=key[sc * 512:(sc + 1) * 512,
                            h * w:(h + 1) * w].rearrange(
                        "(c p) d -> p c d", p=128))
                pieces.append(kin)
            return nsplit, pieces

        kins_all = [k_load(0, nsplit=4)]
        wq_st = const.tile([128, 4, D], BF16)
        nc.gpsimd.dma_start(out=wq_st,
                            in_=wq[:, :].rearrange("(c p) d -> p c d", p=128))

        qin_pool = ctx.enter_context(tc.tile_pool(name="qin", bufs=2))
        qTd_pool = ctx.enter_context(tc.tile_pool(name="qTd", bufs=9))
        qsb_pool = ctx.enter_context(tc.tile_pool(name="qsb", bufs=3))
        pT_pool = ctx.enter_context(tc.tile_pool(name="pT", bufs=4))
        xTs_pool = ctx.enter_context(tc.tile_pool(name="xTs", bufs=4))
        small_pool = ctx.enter_context(tc.tile_pool(name="small", bufs=4))
        kTd_pool = ctx.enter_context(tc.tile_pool(name="kTd", bufs=3))

        def q_load(sc):
            qin = qin_pool.tile([128, 4, D], BF16, tag="qin")
            nc.gpsimd.dma_start(
                out=qin,
                in_=query[sc * 512:(sc + 1) * 512, :].rearrange(
                    "(c p) d -> p c d", p=128))
            return qin

        qins0 = q_load(0)
        for sc in (1, 2, 3):
            kins_all.append(k_load(sc))  # halves

        bqsb = const.tile([128, 4], F32)
        bksb = const.tile([128, 4], F32)
        for mc in range(4):
            nc.sync.dma_start(out=bqsb[:, mc:mc + 1],
                              in_=bq[mc * 128:(mc + 1) * 128].unsqueeze(1))
            nc.sync.dma_start(out=bksb[:, mc:mc + 1],
                              in_=bk[mc * 128:(mc + 1) * 128].unsqueeze(1))

        # persistent tiles
        vplus = const.tile([128, 16, 65], BF16)
        nc.gpsimd.memset(vplus[:, :, 64:65], 16.0)
        wqT = const.tile([128, 8, M], BF16)
        wkT = const.tile([128, 8, M], BF16)
        qT_pair = [const.tile([128, S], BF16, name=f"qTp{p}") for p in range(NPAIR)]
        kT_pair = [const.tile([128, S], BF16, name=f"kTp{p}") for p in range(NPAIR)]
        x_acc = const.tile([128, 16, DK], F32)

        # --- value transpose + Wk transposes (short PE burst, own psum) ---
        # dummy transposes keep the PE busy through the DMA ramp so the HAM
        # clock-gate is warm when real work lands
        with tc.tile_pool(name="warm", bufs=1, space="PSUM") as warm_pool:
            wtile = warm_pool.tile([128, 128], F32, tag="warm")
            for _ in range(40):
                nc.tensor.transpose(wtile, ident_f, ident_f)
        with tc.tile_pool(name="wps", bufs=2, space="PSUM") as wps_pool:
            for g in range(4):  # 4 groups of 4 value chunks
                vps = wps_pool.tile([128, 4, DK], F32, tag="wps", name="vps")
                for j in range(4):
                    jc = g * 4 + j
                    nc.tensor.transpose(vps[:, j, :],
                                        vstage[:, jc * 128:(jc + 1) * 128],
                                        ident_f[0:DK, 0:DK])
                nc.scalar.copy(vplus[:, g * 4:(g + 1) * 4, 0:DK], vps)
            for dc in range(8):
                wpsb = wps_pool.tile([128, 512], BF16, tag="wps", name="wpsb")
                for wmc in range(4):
                    nc.tensor.transpose(wpsb[:, wmc * 128:(wmc + 1) * 128],
                                        wk_st[:, wmc, dc * 128:(dc + 1) * 128],
                                        ident_b)
                nc.scalar.copy(wkT[:, dc, :], wpsb)
            junk = wps_pool.tile([128, 128], BF16, tag="wps", name="junk")
            for rep in range(48):
                nc.tensor.transpose(junk, wk_st[:, 0, 0:128], ident_b)

        # --- misc psum pool shared by Wq/Q-proj staging and the epilogue ---
        misc_pool = ctx.enter_context(
            tc.tile_pool(name="misc", bufs=2, space="PSUM"))

        def wq_item(dc):
            wpsq = misc_pool.tile([128, 512], BF16, tag="misc", name="wpsq")
            for wmc in range(4):
                nc.tensor.transpose(wpsq[:, wmc * 128:(wmc + 1) * 128],
                                    wq_st[:, wmc, dc * 128:(dc + 1) * 128],
                                    ident_b)
            nc.scalar.copy(wqT[:, dc, :], wpsq)

        def q_transpose_group(qins, dc, out_tiles):
            tpq = misc_pool.tile([128, 512], BF16, tag="misc", name="tpq")
            for sj in range(4):
                nc.tensor.transpose(
                    tpq[:, sj * 128:(sj + 1) * 128],
                    qins[:, sj, dc * 128:(dc + 1) * 128], ident_b)
            qTd = qTd_pool.tile([128, 512], BF16, tag="qTd", name="qTd")
            nc.vector.tensor_copy(qTd, tpq)
            out_tiles.append(qTd)

        def q_proj_mm(sc, mc, qTd_tiles, dc, state):
            if dc == 0:
                state[mc] = misc_pool.tile([128, 512], F32, tag="misc",
                                           name="accq")
            nc.tensor.matmul(state[mc],
                             wqT[:, dc, mc * 128:(mc + 1) * 128],
                             qTd_tiles[dc],
                             start=(dc == 0), stop=(dc == 7))

        def q_proj_fin(sc, mc, state, use_act=False):
            accq = state[mc]
            qsb = qsb_pool.tile([128, 512], F32, tag="qsb")
            if use_act:
                nc.scalar.add(qsb, accq, bqsb[:, mc:mc + 1])
            else:
                nc.vector.tensor_scalar_add(qsb, accq, bqsb[:, mc:mc + 1])
            nc.sync.dma_start(
                out=qout[mc * 128:(mc + 1) * 128, sc * 512:(sc + 1) * 512],
                in_=qsb)
            nc.vector.tensor_scalar_add(
                qT_pair[mc][:, sc * 512:(sc + 1) * 512],
                accq, bqsb[:, mc:mc + 1])

        # --- K projection with Wq + Q(sc0) work threaded through ---
        tiles0 = []
        st0 = {}
        prefix_q = deque()
        for dc in range(8):
            prefix_q.append(lambda dc=dc: wq_item(dc))
        for dc in range(8):
            prefix_q.append(lambda dc=dc: q_transpose_group(qins0, dc, tiles0))
        for mc in range(4):
            for dc in range(8):
                prefix_q.append(lambda mc=mc, dc=dc:
                                q_proj_mm(0, mc, tiles0, dc, st0))
            prefix_q.append(lambda mc=mc: q_proj_fin(0, mc, st0, use_act=True))

        with tc.tile_pool(name="ktp", bufs=2, space="PSUM") as ktp_pool, \
             tc.tile_pool(name="kacc", bufs=1, space="PSUM") as kacc_pool:
            for sc in range(4):
                acc = kacc_pool.tile([128, 4, 512], F32, tag="kacc")
                for dc in range(8):
                    tp = ktp_pool.tile([128, 512], BF16, tag="ktp")
                    for sj in range(4):
                        nc.tensor.transpose(
                            tp[:, sj * 128:(sj + 1) * 128],
                            kins_all[sc][1][dc // (8 // kins_all[sc][0])][
                                :, sj,
                                (dc % (8 // kins_all[sc][0])) * 128:
                                (dc % (8 // kins_all[sc][0]) + 1) * 128],
                            ident_b)
                    kT = kTd_pool.tile([128, 512], BF16, tag="kTd")
                    if dc % 2 == 0:
                        nc.scalar.copy(kT, tp)
                    else:
                        nc.vector.tensor_copy(kT, tp)
                    for mc in range(4):
                        nc.tensor.matmul(acc[:, mc, :],
                                         wkT[:, dc, mc * 128:(mc + 1) * 128], kT,
                                         start=(dc == 0), stop=(dc == 7))
                    if prefix_q:
                        prefix_q.popleft()()
                    if sc > 0 and prefix_q:
                        prefix_q.popleft()()
                for mc in range(4):
                    if mc % 2 == 0:
                        nc.scalar.add(kT_pair[mc][:, sc * 512:(sc + 1) * 512],
                                      acc[:, mc, :], bksb[:, mc:mc + 1])
                    else:
                        nc.vector.tensor_scalar_add(
                            kT_pair[mc][:, sc * 512:(sc + 1) * 512],
                            acc[:, mc, :], bksb[:, mc:mc + 1])
            while prefix_q:
                prefix_q.popleft()()

        # ---- attention: sc 4 banks + xA/xB 2 banks (+ misc 2) = 8 ----
        with tc.tile_pool(name="scps", bufs=2, space="PSUM") as sc_pool, \
             tc.tile_pool(name="xps", bufs=1, space="PSUM") as x_pool:

            def normalize_item(xTs, t, ic, is_first):
                xp = misc_pool.tile([128, 512], F32, tag="misc", name="xp")
                nc.tensor.transpose(xp[:, 0:65], xTs[:, t * 128:(t + 1) * 128],
                                    ident_f[0:65, 0:65])
                r = small_pool.tile([128, 1], F32, tag="r")
                nc.vector.reciprocal(r, xp[:, DK:DK + 1])
                tg = ic * 4 + t
                if is_first:
                    nc.vector.tensor_scalar_mul(x_acc[:, tg, :], xp[:, 0:DK], r)
                else:
                    nc.vector.scalar_tensor_tensor(
                        out=x_acc[:, tg, :], in0=xp[:, 0:DK], scalar=r,
                        in1=x_acc[:, tg, :], op0=MUL, op1=ADD)

            def attention(p, ic, side):
                kT = kT_pair[p]
                qT = qT_pair[p]
                xA = x_pool.tile([65, 512], F32, tag="xA")
                xB = x_pool.tile([65, 512], F32, tag="xB")
                for jc in range(16):
                    scps = sc_pool.tile([128, 2, 512], F32, tag="sc")
                    nc.tensor.matmul(scps[:, 0, :],
                                     kT[0:64, jc * 128:(jc + 1) * 128],
                                     qT[0:64, ic * 512:(ic + 1) * 512],
                                     start=True, stop=True)
                    nc.tensor.matmul(scps[:, 1, :],
                                     kT[64:128, jc * 128:(jc + 1) * 128],
                                     qT[64:128, ic * 512:(ic + 1) * 512],
                                     start=True, stop=True)
                    pT = pT_pool.tile([128, 2, 512], BF16, tag="pT")
                    nc.scalar.activation(pT, scps, Exp, scale=0.125)
                    nc.tensor.matmul(xA, vplus[:, jc, :], pT[:, 0, :],
                                     start=(jc == 0), stop=(jc == 15))
                    nc.tensor.matmul(xB, vplus[:, jc, :], pT[:, 1, :],
                                     start=(jc == 0), stop=(jc == 15))
                    for fn in side.get(jc, ()):
                        fn()
                deferred = []
                for a, xps in ((0, xA), (1, xB)):
                    xTs = xTs_pool.tile([65, 512], F32, tag="xTs")
                    nc.vector.tensor_copy(xTs, xps)
                    for t in range(4):
                        deferred.append(
                            lambda x=xTs, t=t, f=(p == 0 and a == 0):
                            normalize_item(x, t, ic, f))
                return deferred

            deferred_q = deque()
            for ic in range(4):
                nsc = ic + 1
                work_q = deque()
                if nsc < 4:
                    qins = q_load(nsc)
                    ntiles = []
                    nst = {}
                    for g in range(8):
                        work_q.append(lambda g=g, q=qins, t=ntiles:
                                      q_transpose_group(q, g, t))
                    for mc in range(4):
                        for dc in range(8):
                            work_q.append(lambda mc=mc, dc=dc, t=ntiles, s=nst:
                                          q_proj_mm(nsc, mc, t, dc, s))
                        work_q.append(lambda mc=mc, s=nst: q_proj_fin(nsc, mc, s))
                for p in range(NPAIR):
                    side = {}
                    for jc in range(16):
                        items = []
                        if work_q:
                            items.append(work_q.popleft())
                        if deferred_q:
                            items.append(deferred_q.popleft())
                        if items:
                            side[jc] = items
                    deferred_q.extend(attention(p, ic, side))
                while work_q:
                    work_q.popleft()()
            while deferred_q:
                deferred_q.popleft()()

        nc.sync.dma_start(out=xout[:, :].rearrange("(t p) e -> p t e", p=128),
                          in_=x_acc)

    nc.finalize()
    return nc


def _get_built():
    global _built
    if _built is None:
        _built = _build()
    return _built


def _make_in_maps(inputs):
    query = np.asarray(inputs["query"], dtype=np.float32)
    key = np.asarray(inputs["key"], dtype=np.float32)
    value = np.asarray(inputs["value"], dtype=np.float32)
    Wq = np.asarray(inputs["Wq"], dtype=np.float32)
    bq = np.asarray(inputs["bq"], dtype=np.float32)
    Wk = np.asarray(inputs["Wk"], dtype=np.float32)
    bk = np.asarray(inputs["bk"], dtype=np.float32)
    in_maps = []
    for c in range(8):
        b, hh = c // 2, c % 2
        sl = slice(hh * M, (hh + 1) * M)
        in_maps.append({
            "query": query[b],
            "key": key[b],
            "value": value[b],
            "wq": np.ascontiguousarray(Wq[sl]),
            "wk": np.ascontiguousarray(Wk[sl]),
            "bq": np.ascontiguousarray(bq[sl]),
            "bk": np.ascontiguousarray(bk[sl]),
        })
    return in_maps


def kernel(query, key, value, Wq, bq, Wk, bk):
    nc = _get_built()
    in_maps = _make_in_maps(dict(query=query, key=key, value=value,
                                 Wq=Wq, bq=bq, Wk=Wk, bk=bk))
    res = run_bass_kernel_spmd(nc, in_maps, list(range(8)))

    B = np.asarray(query).shape[0]
    H = 16
    q_full = np.empty((B, H, S, DK), dtype=np.float32)
    mean_x = np.empty((B, S, DK), dtype=np.float32)
    for c in range(8):
        b, hh = c // 2, c % 2
        r = res.results[c]
        q_full[b, hh * NHEAD:(hh + 1) * NHEAD] = (
            r["qout"].reshape(NHEAD, DK, S).transpose(0, 2, 1))
        if hh == 0:
            mean_x[b] = r["xout"]
        else:
            mean_x[b] += r["xout"]
    return mean_x, q_full


# revision 32
# speedup vs baseline: 1.0444x; 1.0444x over previous
"""Trainium2 Bass kernel for nn_MultiHeadedAttention_53626961658052.

Full-input contract: kernel(**inputs) takes the unsharded numpy inputs and
returns the full outputs (mean_x [4,2048,64], q [4,16,2048,64]) as a tuple,
matching the reference.

Sharding: 8 cores = 4 batches x 2 head-halves. Core c handles batch c//2 and
heads (c%2)*8 .. (c%2)*8+8. Per core:
  - query/key/weights are cast to bf16 and transposed on the PE (the
    contraction dim must sit on SBUF partitions); projections accumulate in
    fp32 PSUM with fp32 bias, so q comes out at ~2e-3;
  - scores^T = k_h^T q_h per head as two K=64 matmuls row-packed into the
    128x128 PE array (tile_position); exp on the scalar engine straight from
    PSUM (scale=1/8 fused; max-subtraction skipped: scores are in [-10, 11]);
  - x^T = [v | 16]^T @ p^T with M=65 matmuls (ones column -> 16*rowsum,
    folding the /16 head-mean into the reciprocal);
  - x^T is transposed back on the PE, normalized and accumulated on DVE.

Scheduling: the kernel is scalar-engine(exp)-bound in steady state. The Q
projection for s-chunk sc+1 and the deferred normalize work are chopped into
small items and threaded through the attention jc-loops; the Q projection for
sc0 and the Wq transposes are likewise threaded through the K projection, so
the serial prefix is just value/Wk staging plus the K projection itself.
"""

import numpy as np

import concourse.bass as bass
import concourse.mybir as mybir
import concourse.tile as tile
from concourse import bacc
from concourse.bass_utils import run_bass_kernel_spmd
from concourse.masks import make_identity
from contextlib import ExitStack
from collections import deque

F32 = mybir.dt.float32
F32R = mybir.dt.float32r
BF16 = mybir.dt.bfloat16
Exp = mybir.ActivationFunctionType.Exp
MUL = mybir.AluOpType.mult
ADD = mybir.AluOpType.add

S = 2048
D = 1024
M = 512          # head-dim columns per core = 8 heads * 64
NHEAD = 8
NPAIR = 4
DK = 64

_built = None


def _build():
    nc = bacc.Bacc(None, target_bir_lowering=False)
    query = nc.dram_tensor("query", [S, D], F32, kind="ExternalInput")
    key = nc.dram_tensor("key", [S, D], F32, kind="ExternalInput")
    value = nc.dram_tensor("value", [DK, S], F32, kind="ExternalInput")
    wq = nc.dram_tensor("wq", [M, D], F32, kind="ExternalInput")
    wk = nc.dram_tensor("wk", [M, D], F32, kind="ExternalInput")
    bq = nc.dram_tensor("bq", [M], F32, kind="ExternalInput")
    bk = nc.dram_tensor("bk", [M], F32, kind="ExternalInput")
    qout = nc.dram_tensor("qout", [M, S], F32, kind="ExternalOutput")
    xout = nc.dram_tensor("xout", [S, DK], F32, kind="ExternalOutput")

    with tile.TileContext(nc) as tc, ExitStack() as ctx:
        const = ctx.enter_context(tc.tile_pool(name="const", bufs=1))

        ident_f = const.tile([128, 128], F32)
        make_identity(nc, ident_f)
        ident_b = const.tile([128, 128], BF16)
        make_identity(nc, ident_b)

        # --- kick off every input load up front ---
        vstage = const.tile([DK, S], F32)
        nc.sync.dma_start(out=vstage, in_=value[:, :])
        wk_st = const.tile([128, 4, D], BF16)
        nc.gpsimd.dma_start(out=wk_st,
                            in_=wk[:, :].rearrange("(c p) d -> p c d", p=128))
        kin_pool = ctx.enter_context(tc.tile_pool(name="kin", bufs=8))

        def k_load(sc):
            halves = []
            for h in range(2):
                kin = kin_pool.tile([128, 4, 512], BF16, tag="kin")
                nc.gpsimd.dma_start(
                    out=kin,
                    in_=key[sc * 512:(sc + 1) * 512,
                            h * 512:(h + 1) * 512].rearrange(
                        "(c p) d -> p c d", p=128))
                halves.append(kin)
            return halves

        kins_all = [k_load(0)]
        wq_st = const.tile([128, 4, D], BF16)
        nc.gpsimd.dma_start(out=wq_st,
                            in_=wq[:, :].rearrange("(c p) d -> p c d", p=128))

        qin_pool = ctx.enter_context(tc.tile_pool(name="qin", bufs=2))
        qTd_pool = ctx.enter_context(tc.tile_pool(name="qTd", bufs=9))
        qsb_pool = ctx.enter_context(tc.tile_pool(name="qsb", bufs=3))
        pT_pool = ctx.enter_context(tc.tile_pool(name="pT", bufs=4))
        xTs_pool = ctx.enter_context(tc.tile_pool(name="xTs", bufs=4))
        small_pool = ctx.enter_context(tc.tile_pool(name="small", bufs=4))
        kTd_pool = ctx.enter_context(tc.tile_pool(name="kTd", bufs=3))

        def q_load(sc):
            qin = qin_pool.tile([128, 4, D], BF16, tag="qin")
            nc.gpsimd.dma_start(
                out=qin,
                in_=query[sc * 512:(sc + 1) * 512, :].rearrange(
                    "(c p) d -> p c d", p=128))
            return qin

        qins0 = q_load(0)
        for sc in (1, 2, 3):
            kins_all.append(k_load(sc))

        bqsb = const.tile([128, 4], F32)
        bksb = const.tile([128, 4], F32)
        for mc in range(4):
            nc.sync.dma_start(out=bqsb[:, mc:mc + 1],
                              in_=bq[mc * 128:(mc + 1) * 128].unsqueeze(1))
            nc.sync.dma_start(out=bksb[:, mc:mc + 1],
                              in_=bk[mc * 128:(mc + 1) * 128].unsqueeze(1))

        # persistent tiles
        vplus = const.tile([128, 16, 65], BF16)
        nc.gpsimd.memset(vplus[:, :, 64:65], 16.0)
        wqT = const.tile([128, 8, M], BF16)
        wkT = const.tile([128, 8, M], BF16)
        qT_pair = [const.tile([128, S], BF16, name=f"qTp{p}") for p in range(NPAIR)]
        kT_pair = [const.tile([128, S], BF16, name=f"kTp{p}") for p in range(NPAIR)]
        x_acc = const.tile([128, 16, DK], F32)

        # --- value transpose + Wk transposes (short PE burst, own psum) ---
        # dummy transposes keep the PE busy through the DMA ramp so the HAM
        # clock-gate is warm when real work lands
        with tc.tile_pool(name="warm", bufs=1, space="PSUM") as warm_pool:
            wtile = warm_pool.tile([128, 128], F32, tag="warm")
            for _ in range(40):
                nc.tensor.transpose(wtile, ident_f, ident_f)
        with tc.tile_pool(name="wps", bufs=2, space="PSUM") as wps_pool:
            for g in range(4):  # 4 groups of 4 value chunks
                vps = wps_pool.tile([128, 4, DK], F32, tag="wps", name="vps")
                for j in range(4):
                    jc = g * 4 + j
                    nc.tensor.transpose(vps[:, j, :],
                                        vstage[:, jc * 128:(jc + 1) * 128],
                                        ident_f[0:DK, 0:DK])
                nc.scalar.copy(vplus[:, g * 4:(g + 1) * 4, 0:DK], vps)
            for dc in range(8):
                wpsb = wps_pool.tile([128, 512], BF16, tag="wps", name="wpsb")
                for wmc in range(4):
                    nc.tensor.transpose(wpsb[:, wmc * 128:(wmc + 1) * 128],
                                        wk_st[:, wmc, dc * 128:(dc + 1) * 128],
                                        ident_b)
                nc.scalar.copy(wkT[:, dc, :], wpsb)
            junk = wps_pool.tile([128, 128], BF16, tag="wps", name="junk")
            for rep in range(16):
                nc.tensor.transpose(junk, wk_st[:, 0, 0:128], ident_b)

        # --- misc psum pool shared by Wq/Q-proj staging and the epilogue ---
        misc_pool = ctx.enter_context(
            tc.tile_pool(name="misc", bufs=2, space="PSUM"))

        def wq_item(dc):
            wpsq = misc_pool.tile([128, 512], BF16, tag="misc", name="wpsq")
            for wmc in range(4):
                nc.tensor.transpose(wpsq[:, wmc * 128:(wmc + 1) * 128],
                                    wq_st[:, wmc, dc * 128:(dc + 1) * 128],
                                    ident_b)
            nc.scalar.copy(wqT[:, dc, :], wpsq)

        def q_transpose_group(qins, dc, out_tiles):
            tpq = misc_pool.tile([128, 512], BF16, tag="misc", name="tpq")
            for sj in range(4):
                nc.tensor.transpose(
                    tpq[:, sj * 128:(sj + 1) * 128],
                    qins[:, sj, dc * 128:(dc + 1) * 128], ident_b)
            qTd = qTd_pool.tile([128, 512], BF16, tag="qTd", name="qTd")
            nc.vector.tensor_copy(qTd, tpq)
            out_tiles.append(qTd)

        def q_proj_mm(sc, mc, qTd_tiles, dc, state):
            if dc == 0:
                state[mc] = misc_pool.tile([128, 512], F32, tag="misc",
                                           name="accq")
            nc.tensor.matmul(state[mc],
                             wqT[:, dc, mc * 128:(mc + 1) * 128],
                             qTd_tiles[dc],
                             start=(dc == 0), stop=(dc == 7))

        def q_proj_fin(sc, mc, state, use_act=False):
            accq = state[mc]
            qsb = qsb_pool.tile([128, 512], F32, tag="qsb")
            if use_act:
                nc.scalar.add(qsb, accq, bqsb[:, mc:mc + 1])
            else:
                nc.vector.tensor_scalar_add(qsb, accq, bqsb[:, mc:mc + 1])
            nc.sync.dma_start(
                out=qout[mc * 128:(mc + 1) * 128, sc * 512:(sc + 1) * 512],
                in_=qsb)
            nc.vector.tensor_scalar_add(
                qT_pair[mc][:, sc * 512:(sc + 1) * 512],
                accq, bqsb[:, mc:mc + 1])

        # --- K projection with Wq + Q(sc0) work threaded through ---
        tiles0 = []
        st0 = {}
        prefix_q = deque()
        for dc in range(8):
            prefix_q.append(lambda dc=dc: wq_item(dc))
        for dc in range(8):
            prefix_q.append(lambda dc=dc: q_transpose_group(qins0, dc, tiles0))
        for mc in range(4):
            for dc in range(8):
                prefix_q.append(lambda mc=mc, dc=dc:
                                q_proj_mm(0, mc, tiles0, dc, st0))
            prefix_q.append(lambda mc=mc: q_proj_fin(0, mc, st0, use_act=True))

        with tc.tile_pool(name="ktp", bufs=2, space="PSUM") as ktp_pool, \
             tc.tile_pool(name="kacc", bufs=1, space="PSUM") as kacc_pool:
            for sc in range(4):
                acc = kacc_pool.tile([128, 4, 512], F32, tag="kacc")
                for dc in range(8):
                    tp = ktp_pool.tile([128, 512], BF16, tag="ktp")
                    for sj in range(4):
                        nc.tensor.transpose(
                            tp[:, sj * 128:(sj + 1) * 128],
                            kins_all[sc][dc // 4][:, sj,
                                                  (dc % 4) * 128:
                                                  (dc % 4 + 1) * 128],
                            ident_b)
                    kT = kTd_pool.tile([128, 512], BF16, tag="kTd")
                    if dc % 2 == 0:
                        nc.scalar.copy(kT, tp)
                    else:
                        nc.vector.tensor_copy(kT, tp)
                    for mc in range(4):
                        nc.tensor.matmul(acc[:, mc, :],
                                         wkT[:, dc, mc * 128:(mc + 1) * 128], kT,
                                         start=(dc == 0), stop=(dc == 7))
                    if prefix_q:
                        prefix_q.popleft()()
                    if sc > 0 and prefix_q:
                        prefix_q.popleft()()
                for mc in range(4):
                    if mc % 2 == 0:
                        nc.scalar.add(kT_pair[mc][:, sc * 512:(sc + 1) * 512],
                                      acc[:, mc, :], bksb[:, mc:mc + 1])
                    else:
                        nc.vector.tensor_scalar_add(
                            kT_pair[mc][:, sc * 512:(sc + 1) * 512],
                            acc[:, mc, :], bksb[:, mc:mc + 1])
            while prefix_q:
                prefix_q.popleft()()

        # ---- attention: sc 4 banks + xA/xB 2 banks (+ misc 2) = 8 ----
        with tc.tile_pool(name="scps", bufs=2, space="PSUM") as sc_pool, \
             tc.tile_pool(name="xps", bufs=1, space="PSUM") as x_pool:

            def normalize_item(xTs, t, ic, is_first):
                xp = misc_pool.tile([128, 512], F32, tag="misc", name="xp")
                nc.tensor.transpose(xp[:, 0:65], xTs[:, t * 128:(t + 1) * 128],
                                    ident_f[0:65, 0:65])
                r = small_pool.tile([128, 1], F32, tag="r")
                nc.vector.reciprocal(r, xp[:, DK:DK + 1])
                tg = ic * 4 + t
                if is_first:
                    nc.vector.tensor_scalar_mul(x_acc[:, tg, :], xp[:, 0:DK], r)
                else:
                    nc.vector.scalar_tensor_tensor(
                        out=x_acc[:, tg, :], in0=xp[:, 0:DK], scalar=r,
                        in1=x_acc[:, tg, :], op0=MUL, op1=ADD)

            def attention(p, ic, side):
                kT = kT_pair[p]
                qT = qT_pair[p]
                xA = x_pool.tile([65, 512], F32, tag="xA")
                xB = x_pool.tile([65, 512], F32, tag="xB")
                for jc in range(16):
                    scps = sc_pool.tile([128, 2, 512], F32, tag="sc")
                    nc.tensor.matmul(scps[:, 0, :],
                                     kT[0:64, jc * 128:(jc + 1) * 128],
                                     qT[0:64, ic * 512:(ic + 1) * 512],
                                     start=True, stop=True)
                    nc.tensor.matmul(scps[:, 1, :],
                                     kT[64:128, jc * 128:(jc + 1) * 128],
                                     qT[64:128, ic * 512:(ic + 1) * 512],
                                     start=True, stop=True)
                    pT = pT_pool.tile([128, 2, 512], BF16, tag="pT")
                    nc.scalar.activation(pT, scps, Exp, scale=0.125)
                    nc.tensor.matmul(xA, vplus[:, jc, :], pT[:, 0, :],
                                     start=(jc == 0), stop=(jc == 15))
                    nc.tensor.matmul(xB, vplus[:, jc, :], pT[:, 1, :],
                                     start=(jc == 0), stop=(jc == 15))
                    for fn in side.get(jc, ()):
                        fn()
                deferred = []
                for a, xps in ((0, xA), (1, xB)):
                    xTs = xTs_pool.tile([65, 512], F32, tag="xTs")
                    nc.vector.tensor_copy(xTs, xps)
                    for t in range(4):
                        deferred.append(
                            lambda x=xTs, t=t, f=(p == 0 and a == 0):
                            normalize_item(x, t, ic, f))
                return deferred

            deferred_q = deque()
            for ic in range(4):
                nsc = ic + 1
                work_q = deque()
                if nsc < 4:
                    qins = q_load(nsc)
                    ntiles = []
                    nst = {}
                    for g in range(8):
                        work_q.append(lambda g=g, q=qins, t=ntiles:
                                      q_transpose_group(q, g, t))
                    for mc in range(4):
                        for dc in range(8):
                            work_q.append(lambda mc=mc, dc=dc, t=ntiles, s=nst:
                                          q_proj_mm(nsc, mc, t, dc, s))
                        work_q.append(lambda mc=mc, s=nst: q_proj_fin(nsc, mc, s))
                for p in range(NPAIR):
                    side = {}
                    for jc in range(16):
                        items = []
                        if work_q:
                            items.append(work_q.popleft())
                        if deferred_q:
                            items.append(deferred_q.popleft())
                        if items:
                            side[jc] = items
                    deferred_q.extend(attention(p, ic, side))
                while work_q:
                    work_q.popleft()()
            while deferred_q:
                deferred_q.popleft()()

        nc.sync.dma_start(out=xout[:, :].rearrange("(t p) e -> p t e", p=128),
                          in_=x_acc)

    nc.finalize()
    return nc


def _get_built():
    global _built
    if _built is None:
        _built = _build()
    return _built


def _make_in_maps(inputs):
    query = np.asarray(inputs["query"], dtype=np.float32)
    key = np.asarray(inputs["key"], dtype=np.float32)
    value = np.asarray(inputs["value"], dtype=np.float32)
    Wq = np.asarray(inputs["Wq"], dtype=np.float32)
    bq = np.asarray(inputs["bq"], dtype=np.float32)
    Wk = np.asarray(inputs["Wk"], dtype=np.float32)
    bk = np.asarray(inputs["bk"], dtype=np.float32)
    in_maps = []
    for c in range(8):
        b, hh = c // 2, c % 2
        sl = slice(hh * M, (hh + 1) * M)
        in_maps.append({
            "query": query[b],
            "key": key[b],
            "value": value[b],
            "wq": np.ascontiguousarray(Wq[sl]),
            "wk": np.ascontiguousarray(Wk[sl]),
            "bq": np.ascontiguousarray(bq[sl]),
            "bk": np.ascontiguousarray(bk[sl]),
        })
    return in_maps


def kernel(query, key, value, Wq, bq, Wk, bk):
    nc = _get_built()
    in_maps = _make_in_maps(dict(query=query, key=key, value=value,
                                 Wq=Wq, bq=bq, Wk=Wk, bk=bk))
    res = run_bass_kernel_spmd(nc, in_maps, list(range(8)))

    B = np.asarray(query).shape[0]
    H = 16
    q_full = np.empty((B, H, S, DK), dtype=np.float32)
    mean_x = np.empty((B, S, DK), dtype=np.float32)
    for c in range(8):
        b, hh = c // 2, c % 2
        r = res.results[c]
        q_full[b, hh * NHEAD:(hh + 1) * NHEAD] = (
            r["qout"].reshape(NHEAD, DK, S).transpose(0, 2, 1))
        if hh == 0:
            mean_x[b] = r["xout"]
        else:
            mean_x[b] += r["xout"]
    return mean_x, q_full


# revision 33
# speedup vs baseline: 1.0528x; 1.0080x over previous
"""Trainium2 Bass kernel for nn_MultiHeadedAttention_53626961658052.

Full-input contract: kernel(**inputs) takes the unsharded numpy inputs and
returns the full outputs (mean_x [4,2048,64], q [4,16,2048,64]) as a tuple,
matching the reference.

Sharding: 8 cores = 4 batches x 2 head-halves. Core c handles batch c//2 and
heads (c%2)*8 .. (c%2)*8+8. Per core:
  - query/key/weights are cast to bf16 and transposed on the PE (the
    contraction dim must sit on SBUF partitions); projections accumulate in
    fp32 PSUM with fp32 bias, so q comes out at ~2e-3;
  - scores^T = k_h^T q_h per head as two K=64 matmuls row-packed into the
    128x128 PE array (tile_position); exp on the scalar engine straight from
    PSUM (scale=1/8 fused; max-subtraction skipped: scores are in [-10, 11]);
  - x^T = [v | 16]^T @ p^T with M=65 matmuls (ones column -> 16*rowsum,
    folding the /16 head-mean into the reciprocal);
  - x^T is transposed back on the PE, normalized and accumulated on DVE.

Scheduling: the kernel is scalar-engine(exp)-bound in steady state. The Q
projection for s-chunk sc+1 and the deferred normalize work are chopped into
small items and threaded through the attention jc-loops; the Q projection for
sc0 and the Wq transposes are likewise threaded through the K projection, so
the serial prefix is just value/Wk staging plus the K projection itself.
"""

import numpy as np

import concourse.bass as bass
import concourse.mybir as mybir
import concourse.tile as tile
from concourse import bacc
from concourse.bass_utils import run_bass_kernel_spmd
from concourse.masks import make_identity
from contextlib import ExitStack
from collections import deque

F32 = mybir.dt.float32
F32R = mybir.dt.float32r
BF16 = mybir.dt.bfloat16
Exp = mybir.ActivationFunctionType.Exp
MUL = mybir.AluOpType.mult
ADD = mybir.AluOpType.add

S = 2048
D = 1024
M = 512          # head-dim columns per core = 8 heads * 64
NHEAD = 8
NPAIR = 4
DK = 64

_built = None


def _build():
    nc = bacc.Bacc(None, target_bir_lowering=False)
    query = nc.dram_tensor("query", [S, D], F32, kind="ExternalInput")
    key = nc.dram_tensor("key", [S, D], F32, kind="ExternalInput")
    value = nc.dram_tensor("value", [DK, S], F32, kind="ExternalInput")
    wq = nc.dram_tensor("wq", [M, D], F32, kind="ExternalInput")
    wk = nc.dram_tensor("wk", [M, D], F32, kind="ExternalInput")
    bq = nc.dram_tensor("bq", [M], F32, kind="ExternalInput")
    bk = nc.dram_tensor("bk", [M], F32, kind="ExternalInput")
    qout = nc.dram_tensor("qout", [M, S], F32, kind="ExternalOutput")
    xout = nc.dram_tensor("xout", [S, DK], F32, kind="ExternalOutput")

    with tile.TileContext(nc) as tc, ExitStack() as ctx:
        const = ctx.enter_context(tc.tile_pool(name="const", bufs=1))

        ident_f = const.tile([128, 128], F32)
        make_identity(nc, ident_f)
        ident_b = const.tile([128, 128], BF16)
        make_identity(nc, ident_b)

        # --- kick off every input load up front ---
        vstage = const.tile([DK, S], F32)
        nc.sync.dma_start(out=vstage, in_=value[:, :])
        wk_st = const.tile([128, 4, D], BF16)
        nc.gpsimd.dma_start(out=wk_st,
                            in_=wk[:, :].rearrange("(c p) d -> p c d", p=128))
        kin_pool = ctx.enter_context(tc.tile_pool(name="kin", bufs=8))

        def k_load(sc):
            halves = []
            for h in range(2):
                kin = kin_pool.tile([128, 4, 512], BF16, tag="kin")
                nc.gpsimd.dma_start(
                    out=kin,
                    in_=key[sc * 512:(sc + 1) * 512,
                            h * 512:(h + 1) * 512].rearrange(
                        "(c p) d -> p c d", p=128))
                halves.append(kin)
            return halves

        kins_all = [k_load(0)]
        wq_st = const.tile([128, 4, D], BF16)
        nc.gpsimd.dma_start(out=wq_st,
                            in_=wq[:, :].rearrange("(c p) d -> p c d", p=128))

        qin_pool = ctx.enter_context(tc.tile_pool(name="qin", bufs=2))
        qTd_pool = ctx.enter_context(tc.tile_pool(name="qTd", bufs=9))
        qsb_pool = ctx.enter_context(tc.tile_pool(name="qsb", bufs=3))
        pT_pool = ctx.enter_context(tc.tile_pool(name="pT", bufs=4))
        xTs_pool = ctx.enter_context(tc.tile_pool(name="xTs", bufs=4))
        small_pool = ctx.enter_context(tc.tile_pool(name="small", bufs=4))
        kTd_pool = ctx.enter_context(tc.tile_pool(name="kTd", bufs=3))

        def q_load(sc):
            qin = qin_pool.tile([128, 4, D], BF16, tag="qin")
            nc.gpsimd.dma_start(
                out=qin,
                in_=query[sc * 512:(sc + 1) * 512, :].rearrange(
                    "(c p) d -> p c d", p=128))
            return qin

        qins0 = q_load(0)
        for sc in (1, 2, 3):
            kins_all.append(k_load(sc))

        bqsb = const.tile([128, 4], F32)
        bksb = const.tile([128, 4], F32)
        for mc in range(4):
            nc.sync.dma_start(out=bqsb[:, mc:mc + 1],
                              in_=bq[mc * 128:(mc + 1) * 128].unsqueeze(1))
            nc.sync.dma_start(out=bksb[:, mc:mc + 1],
                              in_=bk[mc * 128:(mc + 1) * 128].unsqueeze(1))

        # persistent tiles
        vplus = const.tile([128, 16, 65], BF16)
        nc.gpsimd.memset(vplus[:, :, 64:65], 16.0)
        wqT = const.tile([128, 8, M], BF16)
        wkT = const.tile([128, 8, M], BF16)
        qT_pair = [const.tile([128, S], BF16, name=f"qTp{p}") for p in range(NPAIR)]
        kT_pair = [const.tile([128, S], BF16, name=f"kTp{p}") for p in range(NPAIR)]
        x_acc = const.tile([128, 16, DK], F32)

        # --- value transpose + Wk transposes (short PE burst, own psum) ---
        # dummy transposes keep the PE busy through the DMA ramp so the HAM
        # clock-gate is warm when real work lands
        with tc.tile_pool(name="warm", bufs=1, space="PSUM") as warm_pool:
            wtile = warm_pool.tile([128, 128], F32, tag="warm")
            for _ in range(40):
                nc.tensor.transpose(wtile, ident_f, ident_f)
        with tc.tile_pool(name="wps", bufs=2, space="PSUM") as wps_pool:
            for g in range(4):  # 4 groups of 4 value chunks
                vps = wps_pool.tile([128, 4, DK], F32, tag="wps", name="vps")
                for j in range(4):
                    jc = g * 4 + j
                    nc.tensor.transpose(vps[:, j, :],
                                        vstage[:, jc * 128:(jc + 1) * 128],
                                        ident_f[0:DK, 0:DK])
                nc.scalar.copy(vplus[:, g * 4:(g + 1) * 4, 0:DK], vps)
            for dc in range(8):
                wpsb = wps_pool.tile([128, 512], BF16, tag="wps", name="wpsb")
                for wmc in range(4):
                    nc.tensor.transpose(wpsb[:, wmc * 128:(wmc + 1) * 128],
                                        wk_st[:, wmc, dc * 128:(dc + 1) * 128],
                                        ident_b)
                nc.scalar.copy(wkT[:, dc, :], wpsb)
            junk = wps_pool.tile([128, 128], BF16, tag="wps", name="junk")
            for rep in range(16):
                nc.tensor.transpose(junk, wk_st[:, 0, 0:128], ident_b)

        # --- misc psum pool shared by Wq/Q-proj staging and the epilogue ---
        misc_pool = ctx.enter_context(
            tc.tile_pool(name="misc", bufs=2, space="PSUM"))

        def wq_item(dc):
            wpsq = misc_pool.tile([128, 512], BF16, tag="misc", name="wpsq")
            for wmc in range(4):
                nc.tensor.transpose(wpsq[:, wmc * 128:(wmc + 1) * 128],
                                    wq_st[:, wmc, dc * 128:(dc + 1) * 128],
                                    ident_b)
            nc.scalar.copy(wqT[:, dc, :], wpsq)

        def q_transpose_group(qins, dc, out_tiles):
            tpq = misc_pool.tile([128, 512], BF16, tag="misc", name="tpq")
            for sj in range(4):
                nc.tensor.transpose(
                    tpq[:, sj * 128:(sj + 1) * 128],
                    qins[:, sj, dc * 128:(dc + 1) * 128], ident_b)
            qTd = qTd_pool.tile([128, 512], BF16, tag="qTd", name="qTd")
            nc.vector.tensor_copy(qTd, tpq)
            out_tiles.append(qTd)

        def q_proj_mm(sc, mc, qTd_tiles, dc, state):
            if dc == 0:
                state[mc] = misc_pool.tile([128, 512], F32, tag="misc",
                                           name="accq")
            nc.tensor.matmul(state[mc],
                             wqT[:, dc, mc * 128:(mc + 1) * 128],
                             qTd_tiles[dc],
                             start=(dc == 0), stop=(dc == 7))

        def q_proj_fin(sc, mc, state, use_act=False):
            accq = state[mc]
            qsb = qsb_pool.tile([128, 512], F32, tag="qsb")
            if use_act:
                nc.scalar.add(qsb, accq, bqsb[:, mc:mc + 1])
            else:
                nc.vector.tensor_scalar_add(qsb, accq, bqsb[:, mc:mc + 1])
            nc.sync.dma_start(
                out=qout[mc * 128:(mc + 1) * 128, sc * 512:(sc + 1) * 512],
                in_=qsb)
            nc.vector.tensor_scalar_add(
                qT_pair[mc][:, sc * 512:(sc + 1) * 512],
                accq, bqsb[:, mc:mc + 1])

        # --- K projection with Wq + Q(sc0) work threaded through ---
        tiles0 = []
        st0 = {}
        prefix_q = deque()
        for dc in range(8):
            prefix_q.append(lambda dc=dc: wq_item(dc))
        for dc in range(8):
            prefix_q.append(lambda dc=dc: q_transpose_group(qins0, dc, tiles0))
        for mc in range(4):
            for dc in range(8):
                prefix_q.append(lambda mc=mc, dc=dc:
                                q_proj_mm(0, mc, tiles0, dc, st0))
            prefix_q.append(lambda mc=mc: q_proj_fin(0, mc, st0, use_act=True))

        with tc.tile_pool(name="ktp", bufs=2, space="PSUM") as ktp_pool, \
             tc.tile_pool(name="kacc", bufs=1, space="PSUM") as kacc_pool:
            for sc in range(4):
                acc = kacc_pool.tile([128, 4, 512], F32, tag="kacc")
                for dc in range(8):
                    tp = ktp_pool.tile([128, 512], BF16, tag="ktp")
                    for sj in range(4):
                        nc.tensor.transpose(
                            tp[:, sj * 128:(sj + 1) * 128],
                            kins_all[sc][dc // 4][:, sj,
                                                  (dc % 4) * 128:
                                                  (dc % 4 + 1) * 128],
                            ident_b)
                    kT = kTd_pool.tile([128, 512], BF16, tag="kTd")
                    if dc % 2 == 0:
                        nc.scalar.copy(kT, tp)
                    else:
                        nc.vector.tensor_copy(kT, tp)
                    for mc in range(4):
                        nc.tensor.matmul(acc[:, mc, :],
                                         wkT[:, dc, mc * 128:(mc + 1) * 128], kT,
                                         start=(dc == 0), stop=(dc == 7))
                    if prefix_q:
                        prefix_q.popleft()()
                    if sc > 0 and prefix_q:
                        prefix_q.popleft()()
                for mc in range(4):
                    if mc % 2 == 0:
                        nc.scalar.add(kT_pair[mc][:, sc * 512:(sc + 1) * 512],
                                      acc[:, mc, :], bksb[:, mc:mc + 1])
                    else:
                        nc.vector.tensor_scalar_add(
                            kT_pair[mc][:, sc * 512:(sc + 1) * 512],
                            acc[:, mc, :], bksb[:, mc:mc + 1])
            while prefix_q:
                prefix_q.popleft()()

        # ---- attention: sc 4 banks + xA/xB 2 banks (+ misc 2) = 8 ----
        with tc.tile_pool(name="scps", bufs=2, space="PSUM") as sc_pool, \
             tc.tile_pool(name="xps", bufs=1, space="PSUM") as x_pool:

            def normalize_item(xTs, t, ic, is_first):
                xp = misc_pool.tile([128, 512], F32, tag="misc", name="xp")
                nc.tensor.transpose(xp[:, 0:65], xTs[:, t * 128:(t + 1) * 128],
                                    ident_f[0:65, 0:65])
                r = small_pool.tile([128, 1], F32, tag="r")
                nc.vector.reciprocal(r, xp[:, DK:DK + 1])
                tg = ic * 4 + t
                if is_first:
                    nc.vector.tensor_scalar_mul(x_acc[:, tg, :], xp[:, 0:DK], r)
                else:
                    nc.vector.scalar_tensor_tensor(
                        out=x_acc[:, tg, :], in0=xp[:, 0:DK], scalar=r,
                        in1=x_acc[:, tg, :], op0=MUL, op1=ADD)

            def attention(p, ic, side):
                kT = kT_pair[p]
                qT = qT_pair[p]
                xA = x_pool.tile([65, 512], F32, tag="xA")
                xB = x_pool.tile([65, 512], F32, tag="xB")
                for jc in range(16):
                    scps = sc_pool.tile([128, 2, 512], F32, tag="sc")
                    nc.tensor.matmul(scps[:, 0, :],
                                     kT[0:64, jc * 128:(jc + 1) * 128],
                                     qT[0:64, ic * 512:(ic + 1) * 512],
                                     start=True, stop=True)
                    nc.tensor.matmul(scps[:, 1, :],
                                     kT[64:128, jc * 128:(jc + 1) * 128],
                                     qT[64:128, ic * 512:(ic + 1) * 512],
                                     start=True, stop=True)
                    pT = pT_pool.tile([128, 2, 512], BF16, tag="pT")
                    nc.scalar.activation(pT, scps, Exp, scale=0.125)
                    nc.tensor.matmul(xA, vplus[:, jc, :], pT[:, 0, :],
                                     start=(jc == 0), stop=(jc == 15))
                    nc.tensor.matmul(xB, vplus[:, jc, :], pT[:, 1, :],
                                     start=(jc == 0), stop=(jc == 15))
                    for fn in side.get(jc, ()):
                        fn()
                deferred = []
                for a, xps in ((0, xA), (1, xB)):
                    xTs = xTs_pool.tile([65, 512], F32, tag="xTs")
                    nc.vector.tensor_copy(xTs, xps)
                    for t in range(4):
                        deferred.append(
                            lambda x=xTs, t=t, f=(p == 0 and a == 0):
                            normalize_item(x, t, ic, f))
                return deferred

            deferred_q = deque()
            for ic in range(4):
                nsc = ic + 1
                work_q = deque()
                if nsc < 4:
                    qins = q_load(nsc)
                    ntiles = []
                    nst = {}
                    for g in range(8):
                        work_q.append(lambda g=g, q=qins, t=ntiles:
                                      q_transpose_group(q, g, t))
                    for mc in range(4):
                        for dc in range(8):
                            work_q.append(lambda mc=mc, dc=dc, t=ntiles, s=nst:
                                          q_proj_mm(nsc, mc, t, dc, s))
                        work_q.append(lambda mc=mc, s=nst: q_proj_fin(nsc, mc, s))
                for p in range(NPAIR):
                    side = {}
                    # keep jc 14-15 free of side work so the next pair's
                    # scores matmuls issue promptly at the boundary
                    for jc in range(14):
                        items = []
                        if work_q:
                            items.append(work_q.popleft())
                        if deferred_q:
                            items.append(deferred_q.popleft())
                        if items:
                            side[jc] = items
                    deferred_q.extend(attention(p, ic, side))
                while work_q:
                    work_q.popleft()()
            while deferred_q:
                deferred_q.popleft()()

        nc.sync.dma_start(out=xout[:, :].rearrange("(t p) e -> p t e", p=128),
                          in_=x_acc)

    nc.finalize()
    return nc


def _get_built():
    global _built
    if _built is None:
        _built = _build()
    return _built


def _make_in_maps(inputs):
    query = np.asarray(inputs["query"], dtype=np.float32)
    key = np.asarray(inputs["key"], dtype=np.float32)
    value = np.asarray(inputs["value"], dtype=np.float32)
    Wq = np.asarray(inputs["Wq"], dtype=np.float32)
    bq = np.asarray(inputs["bq"], dtype=np.float32)
    Wk = np.asarray(inputs["Wk"], dtype=np.float32)
    bk = np.asarray(inputs["bk"], dtype=np.float32)
    in_maps = []
    for c in range(8):
        b, hh = c // 2, c % 2
        sl = slice(hh * M, (hh + 1) * M)
        in_maps.append({
            "query": query[b],
            "key": key[b],
            "value": value[b],
            "wq": np.ascontiguousarray(Wq[sl]),
            "wk": np.ascontiguousarray(Wk[sl]),
            "bq": np.ascontiguousarray(bq[sl]),
            "bk": np.ascontiguousarray(bk[sl]),
        })
    return in_maps


def kernel(query, key, value, Wq, bq, Wk, bk):
    nc = _get_built()
    in_maps = _make_in_maps(dict(query=query, key=key, value=value,
                                 Wq=Wq, bq=bq, Wk=Wk, bk=bk))
    res = run_bass_kernel_spmd(nc, in_maps, list(range(8)))

    B = np.asarray(query).shape[0]
    H = 16
    q_full = np.empty((B, H, S, DK), dtype=np.float32)
    mean_x = np.empty((B, S, DK), dtype=np.float32)
    for c in range(8):
        b, hh = c // 2, c % 2
        r = res.results[c]
        q_full[b, hh * NHEAD:(hh + 1) * NHEAD] = (
            r["qout"].reshape(NHEAD, DK, S).transpose(0, 2, 1))
        if hh == 0:
            mean_x[b] = r["xout"]
        else:
            mean_x[b] += r["xout"]
    return mean_x, q_full
